# revision 3
# baseline (speedup 1.0000x reference)
"""Trainium2 Bass kernel for RoPE linear attention (no softmax, strict causal).

v4: all-HWDGE big-DMA design.
  - Host ships Q de-interleaved (even/odd feature halves) in fp16,
    partition-major ([bh, p, block, n]) so every DMA descriptor is a 16KB
    contiguous run.  Scores are invariant to a feature permutation shared
    by Q and K (K is Q), so de-interleaving changes nothing downstream.
  - Half-size rope tables [T, N/2] fp16 (feature pairs share a frequency),
    also partition-major; rope = 6 contiguous DVE ops per group.
  - V fp16 partition-major; out staged fp16 and upcast on host.
  - Chunked linear attention, C=256 (2 blocks of 128), groups g=0..7:
      intra: S = QR_g QR_g^T (strict causal via mask, symmetry trick),
      state: M += QR_b^T V_b,  inter: QR_g M_{g-1},  AV: S V.
"""

import math
import os
import sys

import numpy as np

for _p in ("/opt/trn_rl_repo",):
    if _p not in sys.path and os.path.isdir(_p):
        sys.path.insert(0, _p)

THETA = 2 ** 16
B, H, T, N, D = 2, 8, 2048, 1024, 128
NB = T // 128            # 16 t-blocks
NG = NB // 2             # 8 groups of 2 blocks (C=256)
NCHUNK = N // 128        # 8 n-chunks
NH = N // 2              # 512 (de-interleaved half)
NC_COUNT = 8
BH_PER_CORE = (B * H) // NC_COUNT  # 2

_cache = {}


def _pmajor(x128):
    """[T, F] -> partition-major [128, NB, F] (t = a*128 + p)."""
    Tt, F = x128.shape
    return np.ascontiguousarray(
        x128.reshape(NB, 128, F).transpose(1, 0, 2))


def _make_tables_half():
    """Half-width cos/sin tables [T, N/2] fp16, partition-major halves."""
    import jax
    import jax.numpy as jnp

    with jax.default_device(jax.devices("cpu")[0]):
        pos = jnp.floor(jnp.arange(N, dtype=jnp.float32) / 2.0) * 2.0
        freqs = 1.0 / (THETA ** (pos / N)) / (2.0 * math.pi)
        r_phases = jnp.arange(T, dtype=jnp.float32)[:, None] * freqs[None, :]
        ph = (r_phases % 1.0) * (2.0 * math.pi)
        c = np.asarray(jnp.cos(ph))[:, 0::2]     # (T, 512) fp32
        s = np.asarray(jnp.sin(ph))[:, 0::2]
    cpm = _pmajor(c.astype(np.float16))          # [128, 16, 512]
    spm = _pmajor(s.astype(np.float16))
    # split halves (blocks 0-7 / 8-15), flatten to [128, 8*512]
    return ([np.ascontiguousarray(cpm[:, h * 8:(h + 1) * 8].reshape(128, -1))
             for h in range(2)],
            [np.ascontiguousarray(spm[:, h * 8:(h + 1) * 8].reshape(128, -1))
             for h in range(2)])


def _build_nc(repeat=1):
    import concourse.mybir as mybir
    from concourse import bacc
    from concourse.tile import TileContext

    f32 = mybir.dt.float32
    f16 = mybir.dt.float16

    ct_np, st_np = _make_tables_half()
    mask_np = np.triu(np.ones((128, 128), np.float16), 1)  # keep s < t
    ident_np = np.eye(128, dtype=np.float16)

    nc = bacc.Bacc("TRN2", target_bir_lowering=False, debug=False,
                   num_devices=NC_COUNT)
    q = nc.dram_tensor("q", [BH_PER_CORE, 128, NB * N], f16,
                       kind="ExternalInput")
    v = nc.dram_tensor("v", [BH_PER_CORE, 128, NB * D], f16,
                       kind="ExternalInput")
    out = nc.dram_tensor("out", [BH_PER_CORE, 128, NB * D], f16,
                         kind="ExternalOutput")
    ct_dram = [nc.inline_tensor(ct_np[h], name=f"ct{h}") for h in range(2)]
    st_dram = [nc.inline_tensor(st_np[h], name=f"st{h}") for h in range(2)]
    mask_dram = nc.inline_tensor(mask_np, name="mask_tab")
    ident_dram = nc.inline_tensor(ident_np, name="ident_tab")

    with TileContext(nc) as tc:
        with tc.tile_pool(name="const", bufs=1) as cpool, \
             tc.tile_pool(name="work", bufs=1) as pool, \
             tc.tile_pool(name="psT", bufs=2, space="PSUM") as psT, \
             tc.tile_pool(name="psS", bufs=1, space="PSUM") as psS, \
             tc.tile_pool(name="psO", bufs=1, space="PSUM") as psO, \
             tc.tile_pool(name="psM", bufs=1, space="PSUM") as psM:

            def emit():
                mask_sb = cpool.tile([128, 128], f16, name="mask")
                nc.sync.dma_start(out=mask_sb, in_=mask_dram[:, :])
                ident_sb = cpool.tile([128, 128], f16, name="ident")
                nc.sync.dma_start(out=ident_sb, in_=ident_dram[:, :])

                # SBUF-resident raw q (per bh, per half), tables, v
                qraw = [[cpool.tile([128, 8 * N], f16, name=f"qraw{bh}_{h}")
                         for h in range(2)] for bh in range(BH_PER_CORE)]
                ct_sb = [cpool.tile([128, 8 * NH], f16, name=f"ct{h}")
                         for h in range(2)]
                st_sb = [cpool.tile([128, 8 * NH], f16, name=f"st{h}")
                        for h in range(2)]
                vf = [cpool.tile([128, NB * D], f16, name=f"vf{bh}")
                      for bh in range(BH_PER_CORE)]

                # prologue loads: first-half data for both bh, then rest
                nc.sync.dma_start(out=qraw[0][0], in_=q[0][:, 0:8 * N])
                nc.sync.dma_start(out=ct_sb[0], in_=ct_dram[0][:, :])
                nc.sync.dma_start(out=st_sb[0], in_=st_dram[0][:, :])
                nc.sync.dma_start(out=qraw[1][0], in_=q[1][:, 0:8 * N])
                nc.sync.dma_start(out=vf[0], in_=v[0][:, :])
                nc.sync.dma_start(out=vf[1], in_=v[1][:, :])
                nc.sync.dma_start(out=qraw[0][1], in_=q[0][:, 8 * N:])
                nc.sync.dma_start(out=ct_sb[1], in_=ct_dram[1][:, :])
                nc.sync.dma_start(out=st_sb[1], in_=st_dram[1][:, :])
                nc.sync.dma_start(out=qraw[1][1], in_=q[1][:, 8 * N:])

                # M state: long-lived PSUM accumulators (2 banks per bh)
                mps = [psM.tile([128, N], f32, tag=f"m{bh}", name=f"mps{bh}")
                       for bh in range(BH_PER_CORE)]
                m_sb = [[cpool.tile([128, N], f16, name=f"msb{bh}_{i}")
                         for i in range(2)] for bh in range(BH_PER_CORE)]
                ostage = [[cpool.tile([128, 8 * D], f16, name=f"os{bh}_{h}")
                           for h in range(2)] for bh in range(BH_PER_CORE)]

                qr = [[None] * NG for _ in range(BH_PER_CORE)]

                def rope(bh, g):
                    """6 contiguous DVE ops: de-interleaved rope, 2 blocks."""
                    h, al = g // 4, 2 * (g % 4)
                    q3 = qraw[bh][h].rearrange("p (a n) -> p a n", a=8)
                    qe = q3[:, al:al + 2, 0:NH]
                    qo = q3[:, al:al + 2, NH:N]
                    c3 = ct_sb[h].rearrange("p (a n) -> p a n", a=8)[
                        :, al:al + 2, :]
                    s3 = st_sb[h].rearrange("p (a n) -> p a n", a=8)[
                        :, al:al + 2, :]
                    r = pool.tile([128, 2 * N], f16, tag="qr", bufs=4,
                                  name=f"qr{bh}_{g}")
                    qr[bh][g] = r
                    r3 = r.rearrange("p (a n) -> p a n", a=2)
                    re = r3[:, :, 0:NH]
                    ro = r3[:, :, NH:N]
                    t1 = pool.tile([128, 2 * NH], f16, tag="rt1", bufs=2,
                                   name=f"t1_{bh}_{g}")
                    t2 = pool.tile([128, 2 * NH], f16, tag="rt2", bufs=2,
                                   name=f"t2_{bh}_{g}")
                    t13 = t1.rearrange("p (a n) -> p a n", a=2)
                    t23 = t2.rearrange("p (a n) -> p a n", a=2)
                    nc.vector.tensor_mul(out=t13, in0=qe, in1=c3)
                    nc.vector.tensor_mul(out=t23, in0=qo, in1=s3)
                    nc.vector.tensor_sub(out=re, in0=t13, in1=t23)
                    nc.vector.tensor_mul(out=t13, in0=qo, in1=c3)
                    nc.vector.tensor_mul(out=t23, in0=qe, in1=s3)
                    nc.vector.tensor_add(out=ro, in0=t13, in1=t23)

                qrt = [[None] * NG for _ in range(BH_PER_CORE)]
                psx_t = [[None] * NG for _ in range(BH_PER_CORE)]
                pstrip = [[None] * NG for _ in range(BH_PER_CORE)]
                pox_t = [[None] * NG for _ in range(BH_PER_CORE)]

                def phase_transpose(bh, g):
                    """PE transposes of both blocks + one qrt drain each."""
                    r3 = qr[bh][g].rearrange("p (a n) -> p a n", a=2)
                    qrt_g = pool.tile([128, NCHUNK * 256], f16, tag="qrt",
                                      bufs=3, name=f"qrt{bh}_{g}")
                    qrt[bh][g] = qrt_g
                    qrt3 = qrt_g.rearrange("p (c t) -> p c t", c=NCHUNK)
                    for bi in range(2):
                        pt = psT.tile([128, N], f16, tag="pt",
                                      name=f"pt{bh}_{g}_{bi}")
                        for k in range(NCHUNK):
                            nc.tensor.transpose(
                                pt[:, k * 128:(k + 1) * 128],
                                r3[:, bi, k * 128:(k + 1) * 128],
                                ident_sb)
                        nc.scalar.copy(
                            qrt3[:, :, bi * 128:(bi + 1) * 128],
                            pt.rearrange("p (c t) -> p c t", c=NCHUNK))

                def phase_intra(bh, g):
                    # intra scores (one bank: ps0 cols 0:256, ps1 256:384).
                    # start=True clears has_written for the WHOLE bank: only
                    # the first matmul carries it; ps1's first write lands on
                    # cleared bits -> overwrite+set.
                    qrt_g = qrt[bh][g]
                    psx = psS.tile([128, 384], f32, tag="ps",
                                   name=f"psx_{bh}_{g}")
                    psx_t[bh][g] = psx
                    ps0 = psx[:, 0:256]
                    ps1 = psx[:, 256:384]
                    for k in range(NCHUNK):
                        ka = qrt_g[:, k * 256:k * 256 + 128]
                        kfull = qrt_g[:, k * 256:(k + 1) * 256]
                        kb = qrt_g[:, k * 256 + 128:(k + 1) * 256]
                        nc.tensor.matmul(ps0, lhsT=ka, rhs=kfull,
                                         start=(k == 0),
                                         stop=(k == NCHUNK - 1))
                        nc.tensor.matmul(ps1, lhsT=kb, rhs=kb,
                                         start=False, stop=(k == NCHUNK - 1))

                def phase_pdrain(bh, g):
                    """P strip drains (AV lhsT, [s,t] layout fp16)."""
                    psx = psx_t[bh][g]
                    ps0 = psx[:, 0:256]
                    ps1 = psx[:, 256:384]
                    p00 = pool.tile([128, 128], f16, tag="p00", bufs=3,
                                    name=f"p00_{bh}_{g}")
                    p01 = pool.tile([128, 128], f16, tag="p01", bufs=3,
                                    name=f"p01_{bh}_{g}")
                    p11 = pool.tile([128, 128], f16, tag="p11", bufs=3,
                                    name=f"p11_{bh}_{g}")
                    nc.vector.tensor_mul(out=p00, in0=ps0[:, 0:128],
                                         in1=mask_sb)
                    nc.scalar.copy(p01, ps0[:, 128:256])
                    nc.vector.tensor_mul(out=p11, in0=ps1, in1=mask_sb)
                    pstrip[bh][g] = (p00, p01, p11)

                def phase_mm(bh, g):
                    """state + inter + AV matmuls (one pox bank-clear)."""
                    b0, b1 = 2 * g, 2 * g + 1
                    r3 = qr[bh][g].rearrange("p (a n) -> p a n", a=2)
                    qrt_g = qrt[bh][g]
                    p00, p01, p11 = pstrip[bh][g]
                    msb_prev = m_sb[bh][(g + 1) % 2]  # state after g-1
                    v3 = vf[bh].rearrange("p (a d) -> p a d", a=NB)
                    pox = psO.tile([128, 2 * D], f32, tag="po",
                                   name=f"pox_{bh}_{g}")
                    pox_t[bh][g] = pox
                    po0 = pox[:, 0:D]
                    po1 = pox[:, D:2 * D]
                    # state update first (no pdrain dependency; hides the
                    # DVE mask-mul latency). mps spans 2 banks; exactly one
                    # bank-clearing start per bank (k==0 / k==4 of the very
                    # first block); not emitted for the last group.
                    if g < NG - 1:
                        for bi, b in enumerate((b0, b1)):
                            for k in range(NCHUNK):
                                nc.tensor.matmul(
                                    mps[bh][:, k * 128:(k + 1) * 128],
                                    lhsT=r3[:, bi, k * 128:(k + 1) * 128],
                                    rhs=v3[:, b, :],
                                    start=(b == 0 and g == 0 and k % 4 == 0),
                                    stop=(g == NG - 2 and bi == 1),
                                )
                    if g > 0:
                        for k in range(NCHUNK):
                            nc.tensor.matmul(
                                po0,
                                lhsT=qrt_g[:, k * 256:k * 256 + 128],
                                rhs=msb_prev[:, k * 128:(k + 1) * 128],
                                start=(k == 0), stop=False)
                            nc.tensor.matmul(
                                po1,
                                lhsT=qrt_g[:, k * 256 + 128:(k + 1) * 256],
                                rhs=msb_prev[:, k * 128:(k + 1) * 128],
                                start=False, stop=False)
                    nc.tensor.matmul(po0, lhsT=p00, rhs=v3[:, b0, :],
                                     start=(g == 0), stop=True)
                    nc.tensor.matmul(po1, lhsT=p01, rhs=v3[:, b0, :],
                                     start=False, stop=False)
                    nc.tensor.matmul(po1, lhsT=p11, rhs=v3[:, b1, :],
                                     start=False, stop=True)

                def phase_drain(bh, g):
                    """M drain (DVE) + out drain (ACT, fp16 staging)."""
                    if g < NG - 1:
                        nc.vector.tensor_copy(out=m_sb[bh][g % 2],
                                              in_=mps[bh])
                    h, al = g // 4, 2 * (g % 4)
                    st3 = ostage[bh][h].rearrange("p (a d) -> p a d", a=8)
                    nc.scalar.copy(
                        st3[:, al:al + 2, :],
                        pox_t[bh][g].rearrange("p (a d) -> p a d", a=2))

                # prologue rope (2 groups ahead)
                for g0 in range(2):
                    for bh in range(BH_PER_CORE):
                        rope(bh, g0)

                for g in range(NG):
                    for bh in range(BH_PER_CORE):
                        phase_transpose(bh, g)
                    for bh in range(BH_PER_CORE):
                        phase_intra(bh, g)
                    if g + 2 < NG:
                        for bh in range(BH_PER_CORE):
                            rope(bh, g + 2)
                    for bh in range(BH_PER_CORE):
                        phase_pdrain(bh, g)
                    for bh in range(BH_PER_CORE):
                        phase_mm(bh, g)
                        phase_drain(bh, g)
                    if g % 4 == 3:
                        h = g // 4
                        for bh in range(BH_PER_CORE):
                            nc.scalar.dma_start(
                                out=out[bh][:, h * 8 * D:(h + 1) * 8 * D],
                                in_=ostage[bh][h])

            for _rep in range(repeat):
                emit()

    nc.compile()
    return nc


def _get_nc():
    if "nc" not in _cache:
        _cache["nc"] = _build_nc()
    return _cache["nc"]


def _prep_q(Qr):
    """[BH, T, N] fp32 -> de-interleaved fp16 partition-major
    [BH, 128, NB*N]."""
    BHt = Qr.shape[0]
    qp = np.empty((BHt, T, N), np.float16)
    qp[..., :NH] = Qr[..., 0::2]
    qp[..., NH:] = Qr[..., 1::2]
    return np.ascontiguousarray(
        qp.reshape(BHt, NB, 128, N).transpose(0, 2, 1, 3)
    ).reshape(BHt, 128, NB * N)


def _prep_v(Vr):
    BHt = Vr.shape[0]
    vp = Vr.astype(np.float16)
    return np.ascontiguousarray(
        vp.reshape(BHt, NB, 128, D).transpose(0, 2, 1, 3)
    ).reshape(BHt, 128, NB * D)


def kernel(Q, K, V):
    from concourse import bass_utils

    del K  # K is Q by construction
    Qr = _prep_q(np.asarray(Q, np.float32).reshape(B * H, T, N))
    Vr = _prep_v(np.asarray(V, np.float32).reshape(B * H, T, D))

    nc = _get_nc()
    in_maps = []
    for c in range(NC_COUNT):
        lo = c * BH_PER_CORE
        in_maps.append({
            "q": np.ascontiguousarray(Qr[lo:lo + BH_PER_CORE]),
            "v": np.ascontiguousarray(Vr[lo:lo + BH_PER_CORE]),
        })

    res = bass_utils.run_bass_kernel_spmd(
        nc, in_maps, core_ids=list(range(NC_COUNT)),
    )
    _cache["last_result"] = res
    outs = [res.results[c]["out"].reshape(BH_PER_CORE, 128, NB, D)
            for c in range(NC_COUNT)]
    full = np.concatenate(outs, axis=0)          # [BH, 128, NB, D] fp16
    full = full.transpose(0, 2, 1, 3).reshape(B, H, T, D)
    return np.ascontiguousarray(full).astype(np.float32)


# revision 11
# speedup vs baseline: 8.9017x; 8.9017x over previous
"""Trainium2 Bass kernel for RoPE linear attention (no softmax, strict causal).

v4: all-HWDGE big-DMA design.
  - Host ships Q de-interleaved (even/odd feature halves) in fp16,
    partition-major ([bh, p, block, n]) so every DMA descriptor is a 16KB
    contiguous run.  Scores are invariant to a feature permutation shared
    by Q and K (K is Q), so de-interleaving changes nothing downstream.
  - Half-size rope tables [T, N/2] fp16 (feature pairs share a frequency),
    also partition-major; rope = 6 contiguous DVE ops per group.
  - V fp16 partition-major; out staged fp16 and upcast on host.
  - Chunked linear attention, C=256 (2 blocks of 128), groups g=0..7:
      intra: S = QR_g QR_g^T (strict causal via mask, symmetry trick),
      state: M += QR_b^T V_b,  inter: QR_g M_{g-1},  AV: S V.
"""

import math
import os
import sys

import numpy as np

for _p in ("/opt/trn_rl_repo",):
    if _p not in sys.path and os.path.isdir(_p):
        sys.path.insert(0, _p)

THETA = 2 ** 16
B, H, T, N, D = 2, 8, 2048, 1024, 128
NB = T // 128            # 16 t-blocks
NG = NB // 2             # 8 groups of 2 blocks (C=256)
NCHUNK = N // 128        # 8 n-chunks
NH = N // 2              # 512 (de-interleaved half)
NC_COUNT = 8
BH_PER_CORE = (B * H) // NC_COUNT  # 2

_cache = {}


def _pmajor(x128):
    """[T, F] -> partition-major [128, NB, F] (t = a*128 + p)."""
    Tt, F = x128.shape
    return np.ascontiguousarray(
        x128.reshape(NB, 128, F).transpose(1, 0, 2))


def _make_tables_half():
    """Half-width cos/sin tables [T, N/2] fp16, partition-major halves."""
    import jax
    import jax.numpy as jnp

    with jax.default_device(jax.devices("cpu")[0]):
        pos = jnp.floor(jnp.arange(N, dtype=jnp.float32) / 2.0) * 2.0
        freqs = 1.0 / (THETA ** (pos / N)) / (2.0 * math.pi)
        r_phases = jnp.arange(T, dtype=jnp.float32)[:, None] * freqs[None, :]
        ph = (r_phases % 1.0) * (2.0 * math.pi)
        c = np.asarray(jnp.cos(ph))[:, 0::2]     # (T, 512) fp32
        s = np.asarray(jnp.sin(ph))[:, 0::2]
    cpm = _pmajor(c.astype(np.float16))          # [128, 16, 512]
    spm = _pmajor(s.astype(np.float16))
    # split halves (blocks 0-7 / 8-15), flatten to [128, 8*512]
    return ([np.ascontiguousarray(cpm[:, h * 8:(h + 1) * 8].reshape(128, -1))
             for h in range(2)],
            [np.ascontiguousarray(spm[:, h * 8:(h + 1) * 8].reshape(128, -1))
             for h in range(2)])


def _build_nc(repeat=1):
    import concourse.mybir as mybir
    from concourse import bacc
    from concourse.tile import TileContext

    f32 = mybir.dt.float32
    f16 = mybir.dt.float16

    ct_np, st_np = _make_tables_half()
    mask_np = np.triu(np.ones((128, 128), np.float16), 1)  # keep s < t
    ident_np = np.eye(128, dtype=np.float16)

    nc = bacc.Bacc("TRN2", target_bir_lowering=False, debug=False,
                   num_devices=NC_COUNT)
    q = nc.dram_tensor("q", [BH_PER_CORE, 128, NB * N], f16,
                       kind="ExternalInput")
    v = nc.dram_tensor("v", [BH_PER_CORE, 128, NB * D], f16,
                       kind="ExternalInput")
    out = nc.dram_tensor("out", [BH_PER_CORE, 128, NB * D], f16,
                         kind="ExternalOutput")
    ct_dram = [nc.inline_tensor(ct_np[h], name=f"ct{h}") for h in range(2)]
    st_dram = [nc.inline_tensor(st_np[h], name=f"st{h}") for h in range(2)]
    mask_dram = nc.inline_tensor(mask_np, name="mask_tab")
    ident_dram = nc.inline_tensor(ident_np, name="ident_tab")

    with TileContext(nc) as tc:
        with tc.tile_pool(name="const", bufs=1) as cpool, \
             tc.tile_pool(name="work", bufs=1) as pool, \
             tc.tile_pool(name="psT", bufs=2, space="PSUM") as psT, \
             tc.tile_pool(name="psS", bufs=1, space="PSUM") as psS, \
             tc.tile_pool(name="psO", bufs=1, space="PSUM") as psO, \
             tc.tile_pool(name="psM", bufs=1, space="PSUM") as psM:

            def emit():
                mask_sb = cpool.tile([128, 128], f16, name="mask")
                nc.sync.dma_start(out=mask_sb, in_=mask_dram[:, :])
                ident_sb = cpool.tile([128, 128], f16, name="ident")
                nc.sync.dma_start(out=ident_sb, in_=ident_dram[:, :])

                # SBUF-resident raw q (per bh, per half), tables, v
                qraw = [[cpool.tile([128, 8 * N], f16, name=f"qraw{bh}_{h}")
                         for h in range(2)] for bh in range(BH_PER_CORE)]
                ct_sb = [cpool.tile([128, 8 * NH], f16, name=f"ct{h}")
                         for h in range(2)]
                st_sb = [cpool.tile([128, 8 * NH], f16, name=f"st{h}")
                        for h in range(2)]
                vf = [cpool.tile([128, NB * D], f16, name=f"vf{bh}")
                      for bh in range(BH_PER_CORE)]

                # prologue loads: first-half data for both bh, then rest
                nc.sync.dma_start(out=qraw[0][0], in_=q[0][:, 0:8 * N])
                nc.sync.dma_start(out=ct_sb[0], in_=ct_dram[0][:, :])
                nc.sync.dma_start(out=st_sb[0], in_=st_dram[0][:, :])
                nc.sync.dma_start(out=qraw[1][0], in_=q[1][:, 0:8 * N])
                nc.sync.dma_start(out=vf[0], in_=v[0][:, :])
                nc.sync.dma_start(out=vf[1], in_=v[1][:, :])
                nc.sync.dma_start(out=qraw[0][1], in_=q[0][:, 8 * N:])
                nc.sync.dma_start(out=ct_sb[1], in_=ct_dram[1][:, :])
                nc.sync.dma_start(out=st_sb[1], in_=st_dram[1][:, :])
                nc.sync.dma_start(out=qraw[1][1], in_=q[1][:, 8 * N:])

                # M state: long-lived PSUM accumulators (2 banks per bh)
                mps = [psM.tile([128, N], f32, tag=f"m{bh}", name=f"mps{bh}")
                       for bh in range(BH_PER_CORE)]
                m_sb = [[cpool.tile([128, N], f16, name=f"msb{bh}_{i}")
                         for i in range(2)] for bh in range(BH_PER_CORE)]
                ostage = [[cpool.tile([128, 8 * D], f16, name=f"os{bh}_{h}")
                           for h in range(2)] for bh in range(BH_PER_CORE)]

                qr = [[None] * NG for _ in range(BH_PER_CORE)]

                def rope(bh, g):
                    """6 contiguous DVE ops: de-interleaved rope, 2 blocks."""
                    h, al = g // 4, 2 * (g % 4)
                    q3 = qraw[bh][h].rearrange("p (a n) -> p a n", a=8)
                    qe = q3[:, al:al + 2, 0:NH]
                    qo = q3[:, al:al + 2, NH:N]
                    c3 = ct_sb[h].rearrange("p (a n) -> p a n", a=8)[
                        :, al:al + 2, :]
                    s3 = st_sb[h].rearrange("p (a n) -> p a n", a=8)[
                        :, al:al + 2, :]
                    r = pool.tile([128, 2 * N], f16, tag="qr", bufs=4,
                                  name=f"qr{bh}_{g}")
                    qr[bh][g] = r
                    r3 = r.rearrange("p (a n) -> p a n", a=2)
                    re = r3[:, :, 0:NH]
                    ro = r3[:, :, NH:N]
                    t1 = pool.tile([128, 2 * NH], f16, tag="rt1", bufs=2,
                                   name=f"t1_{bh}_{g}")
                    t2 = pool.tile([128, 2 * NH], f16, tag="rt2", bufs=2,
                                   name=f"t2_{bh}_{g}")
                    t13 = t1.rearrange("p (a n) -> p a n", a=2)
                    t23 = t2.rearrange("p (a n) -> p a n", a=2)
                    nc.vector.tensor_mul(out=t13, in0=qe, in1=c3)
                    nc.vector.tensor_mul(out=t23, in0=qo, in1=s3)
                    nc.vector.tensor_sub(out=re, in0=t13, in1=t23)
                    nc.vector.tensor_mul(out=t13, in0=qo, in1=c3)
                    nc.vector.tensor_mul(out=t23, in0=qe, in1=s3)
                    nc.vector.tensor_add(out=ro, in0=t13, in1=t23)

                qrt = [[None] * NG for _ in range(BH_PER_CORE)]
                psx_t = [[None] * NG for _ in range(BH_PER_CORE)]
                pstrip = [[None] * NG for _ in range(BH_PER_CORE)]
                pox_t = [[None] * NG for _ in range(BH_PER_CORE)]

                def phase_transpose(bh, g):
                    """PE transposes of both blocks + one qrt drain each."""
                    r3 = qr[bh][g].rearrange("p (a n) -> p a n", a=2)
                    qrt_g = pool.tile([128, NCHUNK * 256], f16, tag="qrt",
                                      bufs=3, name=f"qrt{bh}_{g}")
                    qrt[bh][g] = qrt_g
                    qrt3 = qrt_g.rearrange("p (c t) -> p c t", c=NCHUNK)
                    for bi in range(2):
                        pt = psT.tile([128, N], f16, tag="pt",
                                      name=f"pt{bh}_{g}_{bi}")
                        for k in range(NCHUNK):
                            nc.tensor.transpose(
                                pt[:, k * 128:(k + 1) * 128],
                                r3[:, bi, k * 128:(k + 1) * 128],
                                ident_sb)
                        nc.scalar.copy(
                            qrt3[:, :, bi * 128:(bi + 1) * 128],
                            pt.rearrange("p (c t) -> p c t", c=NCHUNK))

                def phase_intra(bh, g):
                    # intra scores (one bank: ps0 cols 0:256, ps1 256:384).
                    # start=True clears has_written for the WHOLE bank: only
                    # the first matmul carries it; ps1's first write lands on
                    # cleared bits -> overwrite+set.
                    qrt_g = qrt[bh][g]
                    psx = psS.tile([128, 384], f32, tag="ps",
                                   name=f"psx_{bh}_{g}")
                    psx_t[bh][g] = psx
                    ps0 = psx[:, 0:256]
                    ps1 = psx[:, 256:384]
                    for k in range(NCHUNK):
                        ka = qrt_g[:, k * 256:k * 256 + 128]
                        kfull = qrt_g[:, k * 256:(k + 1) * 256]
                        kb = qrt_g[:, k * 256 + 128:(k + 1) * 256]
                        nc.tensor.matmul(ps0, lhsT=ka, rhs=kfull,
                                         start=(k == 0),
                                         stop=(k == NCHUNK - 1))
                        nc.tensor.matmul(ps1, lhsT=kb, rhs=kb,
                                         start=False, stop=(k == NCHUNK - 1))

                def phase_pdrain(bh, g):
                    """P strip drains (AV lhsT, [s,t] layout fp16)."""
                    psx = psx_t[bh][g]
                    ps0 = psx[:, 0:256]
                    ps1 = psx[:, 256:384]
                    p00 = pool.tile([128, 128], f16, tag="p00", bufs=3,
                                    name=f"p00_{bh}_{g}")
                    p01 = pool.tile([128, 128], f16, tag="p01", bufs=3,
                                    name=f"p01_{bh}_{g}")
                    p11 = pool.tile([128, 128], f16, tag="p11", bufs=3,
                                    name=f"p11_{bh}_{g}")
                    nc.vector.tensor_mul(out=p00, in0=ps0[:, 0:128],
                                         in1=mask_sb)
                    nc.scalar.copy(p01, ps0[:, 128:256])
                    nc.vector.tensor_mul(out=p11, in0=ps1, in1=mask_sb)
                    pstrip[bh][g] = (p00, p01, p11)

                def phase_mm(bh, g):
                    """state + inter + AV matmuls (one pox bank-clear)."""
                    b0, b1 = 2 * g, 2 * g + 1
                    r3 = qr[bh][g].rearrange("p (a n) -> p a n", a=2)
                    qrt_g = qrt[bh][g]
                    p00, p01, p11 = pstrip[bh][g]
                    msb_prev = m_sb[bh][(g + 1) % 2]  # state after g-1
                    v3 = vf[bh].rearrange("p (a d) -> p a d", a=NB)
                    pox = psO.tile([128, 2 * D], f32, tag="po",
                                   name=f"pox_{bh}_{g}")
                    pox_t[bh][g] = pox
                    po0 = pox[:, 0:D]
                    po1 = pox[:, D:2 * D]
                    # state update first (no pdrain dependency; hides the
                    # DVE mask-mul latency). mps spans 2 banks; exactly one
                    # bank-clearing start per bank (k==0 / k==4 of the very
                    # first block); not emitted for the last group.
                    if g < NG - 1:
                        for bi, b in enumerate((b0, b1)):
                            for k in range(NCHUNK):
                                nc.tensor.matmul(
                                    mps[bh][:, k * 128:(k + 1) * 128],
                                    lhsT=r3[:, bi, k * 128:(k + 1) * 128],
                                    rhs=v3[:, b, :],
                                    start=(b == 0 and g == 0 and k % 4 == 0),
                                    stop=(g == NG - 2 and bi == 1),
                                )
                    if g > 0:
                        for k in range(NCHUNK):
                            nc.tensor.matmul(
                                po0,
                                lhsT=qrt_g[:, k * 256:k * 256 + 128],
                                rhs=msb_prev[:, k * 128:(k + 1) * 128],
                                start=(k == 0), stop=False)
                            nc.tensor.matmul(
                                po1,
                                lhsT=qrt_g[:, k * 256 + 128:(k + 1) * 256],
                                rhs=msb_prev[:, k * 128:(k + 1) * 128],
                                start=False, stop=False)
                    nc.tensor.matmul(po0, lhsT=p00, rhs=v3[:, b0, :],
                                     start=(g == 0), stop=True)
                    nc.tensor.matmul(po1, lhsT=p01, rhs=v3[:, b0, :],
                                     start=False, stop=False)
                    nc.tensor.matmul(po1, lhsT=p11, rhs=v3[:, b1, :],
                                     start=False, stop=True)

                def phase_drain(bh, g):
                    """M drain (DVE) + out drain (ACT, fp16 staging)."""
                    if g < NG - 1:
                        nc.vector.tensor_copy(out=m_sb[bh][g % 2],
                                              in_=mps[bh])
                    h, al = g // 4, 2 * (g % 4)
                    st3 = ostage[bh][h].rearrange("p (a d) -> p a d", a=8)
                    nc.scalar.copy(
                        st3[:, al:al + 2, :],
                        pox_t[bh][g].rearrange("p (a d) -> p a d", a=2))

                # prologue rope (2 groups ahead)
                for g0 in range(2):
                    for bh in range(BH_PER_CORE):
                        rope(bh, g0)

                for g in range(NG):
                    for bh in range(BH_PER_CORE):
                        phase_transpose(bh, g)
                    for bh in range(BH_PER_CORE):
                        phase_intra(bh, g)
                    if g + 2 < NG:
                        for bh in range(BH_PER_CORE):
                            rope(bh, g + 2)
                    for bh in range(BH_PER_CORE):
                        phase_pdrain(bh, g)
                    for bh in range(BH_PER_CORE):
                        phase_mm(bh, g)
                        phase_drain(bh, g)
                    if g % 4 == 3:
                        h = g // 4
                        for bh in range(BH_PER_CORE):
                            nc.scalar.dma_start(
                                out=out[bh][:, h * 8 * D:(h + 1) * 8 * D],
                                in_=ostage[bh][h])

            for _rep in range(repeat):
                emit()

    nc.compile()
    return nc


def _get_nc():
    if "nc" not in _cache:
        _cache["nc"] = _build_nc()
    return _cache["nc"]


def _prep_q(Qr):
    """[BH, T, N] fp32 -> de-interleaved fp16 partition-major
    [BH, 128, NB*N]."""
    BHt = Qr.shape[0]
    qp = np.empty((BHt, T, N), np.float16)
    qp[..., :NH] = Qr[..., 0::2]
    qp[..., NH:] = Qr[..., 1::2]
    return np.ascontiguousarray(
        qp.reshape(BHt, NB, 128, N).transpose(0, 2, 1, 3)
    ).reshape(BHt, 128, NB * N)


def _prep_v(Vr):
    BHt = Vr.shape[0]
    vp = Vr.astype(np.float16)
    return np.ascontiguousarray(
        vp.reshape(BHt, NB, 128, D).transpose(0, 2, 1, 3)
    ).reshape(BHt, 128, NB * D)


def kernel(Q, K, V):
    from concourse import bass_utils

    del K  # K is Q by construction
    Qr = _prep_q(np.asarray(Q, np.float32).reshape(B * H, T, N))
    Vr = _prep_v(np.asarray(V, np.float32).reshape(B * H, T, D))

    nc = _get_nc()
    in_maps = []
    for c in range(NC_COUNT):
        lo = c * BH_PER_CORE
        in_maps.append({
            "q": np.ascontiguousarray(Qr[lo:lo + BH_PER_CORE]),
            "v": np.ascontiguousarray(Vr[lo:lo + BH_PER_CORE]),
        })

    res = bass_utils.run_bass_kernel_spmd(
        nc, in_maps, core_ids=list(range(NC_COUNT)),
    )
    _cache["last_result"] = res
    outs = [res.results[c]["out"].reshape(BH_PER_CORE, 128, NB, D)
            for c in range(NC_COUNT)]
    full = np.concatenate(outs, axis=0)          # [BH, 128, NB, D] fp16
    full = full.transpose(0, 2, 1, 3).reshape(B, H, T, D)
    return np.ascontiguousarray(full).astype(np.float32)
